# revision 65
# baseline (speedup 1.0000x reference)
"""Bar-level attention Trainium2 kernel (8 NeuronCores, head-parallel).

Contract: kernel(**inputs) takes the FULL inputs from setup_inputs() and
returns the FULL [1, 2048, 512] float32 output.

Strategy (one head per core, 8 heads / 8 cores), all matmuls bf16:
  - Host: transpose hidden -> XT [512, 2048] bf16 (shipped as [128, 4*2048]
    chunk-major); per-head weight pack [128, 4*192] (WqT*scale | WkT | WvT
    per 128-row chunk); bar ids as bf16 (exact: ids < 64): BPQ [128, 2048]
    (row-broadcast) and BPK [128, 16] (chunk-major); consts carry biases and
    1/g, 1/(1-g).
  - Device (per core):
      XT -> Q^T, K^T [64, 2048] (dh on partitions) and V [128, 66] per key
      chunk (natural [k, dh] via stationary-XT matmuls; col 64 = 1/g,
      col 65 = 1/(1-g) -- the sigmoid gate folded into the softmax
      denominator columns).
      Scores S^T = K_c @ Q^T per (chunk, half) -> exp on Act -> E^T tiles
      [128k, 2048q] bf16 in SBUF.  Local masked tiles EL = (BPQ == bpk_c) * E
      via one fused DVE scalar_tensor_tensor over the 128-aligned bar band
      (no mask DMA, no explicit zeroing -- the bar equality does it all).
      AV flipped: for each 128-query chunk, AVT[128q, 66] = sum_c
      (E_c[:, qslice] as stationary) @ V_c -- denominators land on col
      64/65 PER PARTITION, so normalization + gating is 3 small per-
      partition DVE ops -> combined [128, 64] bf16.
      PE-transpose combined -> [64, 128], single output projection per
      q-chunk through Wo_h slice -> out rows [128, 512] f32, DMA'd straight
      from PSUM to DRAM per contiguous 128-row q-chunk (no staging copy).
  - Host: sum the 8 f32 partial outputs (output projection is sharded over
    heads) + bo -> [1, 2048, 512] f32.

The global-attention additive bias in the reference is per-query (constant
across keys), and softmax is shift-invariant per row, so it drops out
exactly; global attention is plain dense softmax attention.
"""

import numpy as np

S = 2048
D = 512
H = 8
DH = 64
SCALE = 1.0 / np.sqrt(DH)
NCHUNK = S // 128       # 16 key chunks of 128
NQ = S // 128           # 16 query chunks of 128
NHALF = 2
QHALF = S // NHALF


def _legalize_waits(nc, mybir):
    """This walrus codegen accepts at most ONE sync wait per instruction.
    Split any instruction carrying N>1 waits into N-1 preceding single-wait
    NoOps on the same engine (waits execute in order on the sequencer)."""
    ctr = 0
    for f in nc.m.functions:
        for b in f.blocks:
            insts = b.instructions
            if not any(i.sync_info and len(i.sync_info.on_wait) > 1 for i in insts):
                continue
            new = []
            for ins in insts:
                si = ins.sync_info
                if si is not None and len(si.on_wait) > 1:
                    waits = list(si.on_wait)
                    for w in waits[:-1]:
                        ctr += 1
                        nop = mybir.InstNoOp(name=f"waitsplit-{ctr}", engine=ins.engine)
                        nop.sync_info = mybir.SyncInfo(on_wait=[w], on_update=[])
                        new.append(nop)
                    ins.sync_info = mybir.SyncInfo(
                        on_wait=[waits[-1]], on_update=list(si.on_update)
                    )
                new.append(ins)
            insts.clear()
            insts.extend(new)
    return ctr


def _bar_bounds(bp):
    """bp: sorted int array [S] -> list of (start, end) per bar."""
    change = np.nonzero(np.diff(bp))[0] + 1
    starts = np.concatenate([[0], change])
    ends = np.concatenate([change, [len(bp)]])
    return list(zip(starts.tolist(), ends.tolist()))


def _bands(bars):
    """Per key-chunk: actual bar-union span and its 128-aligned hull."""
    band, albo, albi = [], [], []
    for c in range(NCHUNK):
        klo, khi = c * 128, (c + 1) * 128
        bs = [b for b in bars if b[1] > klo and b[0] < khi]
        blo, bhi = bs[0][0], bs[-1][1]
        band.append((blo, bhi))
        albo.append((blo // 128) * 128)
        albi.append(-(-bhi // 128) * 128)
    return band, albo, albi


def _build(bars):
    import concourse.bass as bass
    import concourse.tile as tile
    import concourse.mybir as mybir

    dt = mybir.dt
    AF = mybir.ActivationFunctionType
    OP = mybir.AluOpType
    f32 = dt.float32
    bf16 = dt.bfloat16

    band, albo, albi = _bands(bars)
    for c in range(NCHUNK):
        assert albi[c] - albo[c] <= 1024, "bar band too wide for baked kernel"
    # query segments: small first (early exp start), small last (short
    # exposed tail); exp cost is identical to a 1024/1024 split
    SEGS = [(0, 512), (512, 1024), (1536, 512)]
    # el tile of chunk c is computable once exp of segment el_seg[c] is done
    el_seg = [next(s for s, (lo, w) in enumerate(SEGS)
                   if albi[c] <= lo + w) for c in range(NCHUNK)]
    # chunks whose local band misses seg1 entirely: their seg1 E feeds ONLY
    # the global path, where Schraudolph fast-exp is numerically safe
    # (6.1e-3 vs the 2e-2 budget) -- run those exps on DVE as one
    # tensor_scalar (i16 = s*128*log2e + B, bitcast bf16), offloading the
    # saturated Act stream onto DVE slack
    fast1 = {c for c in range(NCHUNK) if albi[c] <= 512 or albo[c] >= 1536}
    # q-chunk -> key chunks with bar overlap
    qcl = [[] for _ in range(NQ)]
    for c in range(NCHUNK):
        blo, bhi = band[c]
        for j in range(blo // 128, (bhi - 1) // 128 + 1):
            qcl[j].append(c)

    nc = bass.Bass()
    # chunk-major packed inputs (single full-speed DMAs)
    xt_d = nc.dram_tensor("xt", [128, 4 * S], bf16, kind="ExternalInput")
    wpack_d = nc.dram_tensor("wpack", [128, 4 * 192], bf16, kind="ExternalInput")
    wot_d = nc.dram_tensor("wot", [DH, D], bf16, kind="ExternalInput")
    ident_d = nc.dram_tensor("ident", [128, 128], bf16, kind="ExternalInput")
    # band-limited bar-equality masks, chunk-major: chunk c occupies
    # cols [moff[c], moff[c+1]) matching its aligned band
    moff = [0]
    for c in range(NCHUNK):
        moff.append(moff[-1] + (albi[c] - albo[c]))
    mask_d = nc.dram_tensor("maskband", [128, moff[-1]], bf16,
                            kind="ExternalInput")
    # consts f32 [128, 4]: col0 bq*SCALE (rows 0:64), col1 bk, col2 1/g,
    # col3 1/(1-g)
    consts_d = nc.dram_tensor("consts", [128, 4], f32, kind="ExternalInput")
    out_d = nc.dram_tensor("out_partial", [S, D], bf16, kind="ExternalOutput")

    with tile.TileContext(nc, pool_alloc_mode="queue") as tc:
        with (
            tc.tile_pool(name="persist", bufs=1) as p_keep,
            tc.tile_pool(name="pr", bufs=4) as p_r,
            tc.tile_pool(name="pt1", bufs=4) as p_t1,
            tc.tile_pool(name="pcb", bufs=4) as p_cb,
            tc.tile_pool(name="pct", bufs=4) as p_ct,
        ):
            qt = p_keep.tile([DH, S], bf16, tag="qt")
            kt = p_keep.tile([DH, S], bf16, tag="kt")
            vt = [p_keep.tile([128, 66], bf16, tag=f"vt{c}", name=f"vt{c}")
                  for c in range(NCHUNK)]
            # per-segment E storage: nothing writes a segment's tiles after
            # its own exps, so there is no cross-segment WAR coarsening.  The
            # 512-wide segments store chunk PAIRS in one [128, 1024] tile so
            # a single exp instruction covers two chunks (halves Act access
            # overhead there).
            ec0 = [p_keep.tile([128, 1024], bf16, tag=f"ec0_{p}",
                               name=f"ec0_{p}") for p in range(NCHUNK // 2)]
            ec1 = [p_keep.tile([128, 1024], bf16, tag=f"ec1_{c}",
                               name=f"ec1_{c}") for c in range(NCHUNK)]
            ec2p = [p_keep.tile([128, 1024], bf16, tag=f"ec2_{p}",
                                name=f"ec2_{p}") for p in range(NCHUNK // 2)]

            def ecs(c, lo, hi):
                if lo >= 512 and hi <= 1536:
                    return ec1[c][:, lo - 512 : hi - 512]
                if hi <= 512:
                    o = (c % 2) * 512
                    return ec0[c // 2][:, o + lo : o + hi]
                assert lo >= 1536
                o = (c % 2) * 512
                return ec2p[c // 2][:, o + lo - 1536 : o + hi - 1536]
            el = [p_keep.tile([128, albi[c] - albo[c]], bf16, tag=f"el{c}",
                              name=f"el{c}")
                  for c in range(NCHUNK)]
            wot = p_keep.tile([DH, D], bf16, tag="wot")
            ident = p_keep.tile([128, 128], bf16, tag="ident")
            consts = p_keep.tile([128, 4], f32, tag="consts")
            outbuf = p_keep.tile([128, NQ * D], bf16, tag="outbuf")
            wzero = p_keep.tile([128, 128], bf16, tag="wzero")
            maskt = p_keep.tile([128, moff[-1]], bf16, tag="maskt")

            # B-phase stages, software-pipelined with a 2-group skew so PE
            # never waits on the DVE normalize chain or the Pool ct copy
            avt_t, cb_t, trf_t, ct_t, outp_t = {}, {}, {}, {}, {}

            def av_global_pair(j, c0, c1, start, stop):
                # tail q-chunks: global AV accumulated pair-by-pair as the
                # seg2 exps land, so only local+normalize remains at the end
                avt = avt_t[j]
                nc.tensor.matmul(avt[:, 66:132], ecs(c0, j * 128, (j + 1) * 128),
                                 vt[c0][:], start=start, stop=False,
                                 skip_group_check=True)
                nc.tensor.matmul(avt[:, 66:132], ecs(c1, j * 128, (j + 1) * 128),
                                 vt[c1][:], start=False, stop=stop,
                                 skip_group_check=True)

            def av_front(j, pool_av, local_first=False, skip_global=False):
                if skip_global:
                    avt = avt_t[j]
                else:
                    avt = pool_av.tile([128, 256], f32, tag=pool_av.name,
                                       name=f"avt{j}")
                    avt_t[j] = avt
                cl = qcl[j]
                r = p_r.tile([128, 2], f32, tag="r", name=f"r{j}")
                t1 = p_t1.tile([128, DH], bf16, tag="t1", name=f"t1{j}")

                def local_mm():
                    for idx, c in enumerate(cl):
                        o = j * 128 - albo[c]
                        nc.tensor.matmul(
                            avt[:, 0:65],
                            el[c][:, o : o + 128],
                            vt[c][:, 0:65],
                            start=(idx == 0),
                            stop=(idx == len(cl) - 1),
                            skip_group_check=True,
                        )

                def global_mm():
                    for c in range(NCHUNK):
                        nc.tensor.matmul(
                            avt[:, 66:132],
                            ecs(c, j * 128, (j + 1) * 128),
                            vt[c][:],
                            start=(c == 0),
                            stop=(c == NCHUNK - 1),
                            skip_group_check=True,
                        )

                def t1_mul():
                    nc.vector.tensor_scalar_mul(t1[:], avt[:, 0:DH],
                                                r[:, 0:1])

                if skip_global:
                    # global already accumulated pair-wise during seg2
                    local_mm()
                    nc.vector.reciprocal(r[:], avt[:, 64:132:67])
                    t1_mul()
                elif local_first:
                    # tail chunks: local normalize runs before the last
                    # global exp lands, shortening the exposed chain
                    local_mm()
                    nc.vector.reciprocal(r[:, 0:1], avt[:, 64:65])
                    t1_mul()
                    global_mm()
                    nc.vector.reciprocal(r[:, 1:2], avt[:, 131:132])
                else:
                    global_mm()
                    local_mm()
                    # one strided recip covers both denominators
                    nc.vector.reciprocal(r[:], avt[:, 64:132:67])
                    t1_mul()
                cb = p_cb.tile([128, DH], bf16, tag="cb", name=f"cb{j}")
                nc.vector.scalar_tensor_tensor(
                    cb[:], avt[:, 66 : 66 + DH], r[:, 1:2], t1[:],
                    OP.mult, OP.add,
                )
                cb_t[j] = cb

            def av_tr(j, pool_tr):
                # transpose [128q, 64dh] -> [64, 128] for the out-proj lhsT
                trf = pool_tr.tile([DH, 64], f32, tag=pool_tr.name,
                                   name=f"tr{j}")
                trp = trf[:].bitcast(bf16)
                nc.tensor.transpose(trp, cb_t[j][:], ident[:])
                ct = p_ct.tile([DH, 128], bf16, tag="ct", name=f"ct{j}")
                nc.vector.tensor_copy(ct[:], trp)
                ct_t[j] = ct

            def av_out(j, pool_op):
                outp = pool_op.tile([128, D], f32, tag=pool_op.name,
                                    name=f"outp{j}")
                nc.tensor.matmul(outp[:], ct_t[j][:], wot[:],
                                 start=True, stop=True)
                # stage bf16 in SBUF (Act takes the tail half; GPSIMD cannot
                # read PSUM so the rest goes to DVE)
                if j >= 6:
                    nc.scalar.copy(outbuf[:, j * D : (j + 1) * D], outp[:])
                else:
                    nc.vector.tensor_copy(outbuf[:, j * D : (j + 1) * D],
                                          outp[:])
                if j >= 8:
                    nc.sync.dma_start(
                        out_d[j * 128 : (j + 1) * 128, :],
                        outbuf[:, j * D : (j + 1) * D],
                    )
                elif j % 4 == 3:
                    j0 = j - 3
                    nc.sync.dma_start(
                        out_d[j0 * 128 : (j + 1) * 128, :].rearrange(
                            "(j p) c -> p j c", j=4
                        ),
                        outbuf[:, j0 * D : (j + 1) * D].rearrange(
                            "p (j c) -> p j c", j=4
                        ),
                    )

            # ------------- section 1: load, proj, exp stream, B(half 0) ----
            with (
                tc.tile_pool(name="inp", bufs=1) as p_in,
                tc.tile_pool(name="ps", bufs=2, space="PSUM") as p_s,
                tc.tile_pool(name="scr", bufs=4, space="PSUM") as p_scr,
            ):
                xts = p_in.tile([128, 4 * S], bf16, tag="xts")
                wps = p_in.tile([128, 4 * 192], bf16, tag="wps")
                # NOTE: no PE p-state warmup needed -- the cost model's ramp
                # is purely time-based (pe_busy_start stays 0), so matmuls
                # are warm after t=3us whether or not PE spun beforehand

                def panel(n):
                    nc.sync.dma_start(
                        xts[:].rearrange("p (i q) -> p i q", i=4)[
                            :, :, n * 512 : (n + 1) * 512
                        ],
                        xt_d[:].rearrange("p (i q) -> p i q", i=4)[
                            :, :, n * 512 : (n + 1) * 512
                        ],
                    )

                nc.sync.dma_start(wps[:], wpack_d[:])
                # panel 0 in two halves: fewer HWDGE dispatches ahead of
                # panel 1 (whose arrival gates the seg0 c=4..7 scores)
                for i in range(2):
                    nc.sync.dma_start(
                        xts[:].rearrange("p (i q) -> p i q", i=4)[
                            :, 2 * i : 2 * i + 2, 0:512
                        ],
                        xt_d[:].rearrange("p (i q) -> p i q", i=4)[
                            :, 2 * i : 2 * i + 2, 0:512
                        ],
                    )
                for n in range(1, 4):
                    panel(n)
                nc.sync.dma_start(consts[:], consts_d[:])
                nc.sync.dma_start(ident[:], ident_d[:])
                nc.sync.dma_start(wot[:], wot_d[:])
                nc.sync.dma_start(maskt[:], mask_d[:])

                def xchunk(i, lo, hi):
                    return xts[:, i * S + lo : i * S + hi]

                def wchunk(i, lo, hi):
                    return wps[:, i * 192 + lo : i * 192 + hi]

                def proj_group(dest, wcol, bcol, hq, n, nm, act=False, lo0=0,
                               hi0=512):
                    w = hi0 - lo0
                    ps = p_scr.tile([DH, w], f32, tag="scr",
                                    name=f"pj_{nm}_{n}")
                    for kc in range(4):
                        nc.tensor.matmul(
                            ps[:],
                            wchunk(kc, wcol, wcol + 64),
                            xchunk(kc, hq * QHALF + n * 512 + lo0,
                                   hq * QHALF + n * 512 + hi0),
                            start=(kc == 0),
                            stop=(kc == 3),
                        )
                    # psum f32 -> sbuf bf16 + per-partition bias add (DVE;
                    # GPSIMD cannot access PSUM on TRN2)
                    lo = hq * QHALF + n * 512 + lo0
                    # bq/bk are zero (usable-check guarded): plain casts,
                    # no consts dependency on the startup chain
                    if act:
                        nc.scalar.copy(dest[:, lo : lo + w], ps[:])
                    else:
                        nc.vector.tensor_copy(dest[:, lo : lo + w], ps[:])

                def proj_half(dest, wcol, bcol, hq, nm):
                    for n in range(2):
                        proj_group(dest, wcol, bcol, hq, n, nm)

                def v_chunk(c):
                    pv = p_scr.tile([128, DH], f32, tag="scr", name=f"pv{c}")
                    for kc in range(4):
                        nc.tensor.matmul(
                            pv[:],
                            xchunk(kc, c * 128, (c + 1) * 128),
                            wchunk(kc, 128, 192),
                            start=(kc == 0),
                            stop=(kc == 3),
                        )
                    nc.vector.tensor_copy(vt[c][:, 0:DH], pv[:])
                    # gate recips into the denominator columns (f32 -> bf16)
                    nc.gpsimd.tensor_copy(vt[c][:, DH : DH + 2], consts[:, 2:4])

                def emit_el(s, cc):
                    # EL = barmask * E over this segment's slice of the band:
                    # plain TensorTensor.  GPSIMD (idle) takes the slack-rich
                    # early pieces; seg2 pieces sit on the tail critical
                    # path, so they go to DVE
                    qlo, w = SEGS[s]
                    plo = max(albo[cc], qlo)
                    phi = min(albi[cc], qlo + w)
                    if plo >= phi:
                        return
                    eng = nc.vector if s == 2 else nc.gpsimd
                    eng.tensor_mul(
                        el[cc][:, plo - albo[cc] : phi - albo[cc]],
                        maskt[:, moff[cc] + plo - albo[cc]
                              : moff[cc] + phi - albo[cc]],
                        ecs(cc, plo, phi),
                    )

                pair_ps = [None]

                def scores_exp(s, c):
                    qlo, w = SEGS[s]
                    # seg2 runs in a PE-oversubscribed window: hint the
                    # scheduler to order its score matmuls ahead of co-ready
                    # pipeline work so the exp stream never starves
                    import contextlib
                    prio = (tc.high_priority(24) if s == 2
                            else contextlib.nullcontext())
                    if w == 512:
                        # chunk pair shares one [128, 1024] psum tile and one
                        # exp into the pair's contiguous ec tile
                        if c % 2 == 0:
                            pair_ps[0] = p_s.tile([128, 1024], f32, tag="s",
                                                  name=f"s{s}_{c}")
                        ps = pair_ps[0]
                        o = (c % 2) * 512
                        with prio:
                            nc.tensor.matmul(
                                ps[:, o : o + 512],
                                kt[:, c * 128 : (c + 1) * 128],
                                qt[:, qlo : qlo + 512],
                                start=True,
                                stop=True,
                            )
                        if c % 2 == 1:
                            dest = ec0[c // 2] if s == 0 else ec2p[c // 2]
                            nc.scalar.activation(dest[:], ps[:], AF.Exp)
                            emit_el(s, c - 1)
                            emit_el(s, c)
                    elif s == 1 and c in fast1:
                        # DVE fast-exp chunks use p_scr half-tiles so the p_s
                        # ring stays an Act-only double buffer (each Act
                        # chunk's scores then prefetch during the PREVIOUS
                        # Act exp instead of waiting on its slot)
                        for n in range(2):
                            psh = p_scr.tile([128, 512], f32, tag="scr",
                                             name=f"s1f_{c}_{n}")
                            nc.tensor.matmul(
                                psh[:],
                                kt[:, c * 128 : (c + 1) * 128],
                                qt[:, qlo + n * 512 : qlo + (n + 1) * 512],
                                start=True,
                                stop=True,
                            )
                            # bf16 Schraudolph: exp via exponent-field affine
                            nc.vector.tensor_scalar(
                                ecs(c, qlo + n * 512,
                                    qlo + (n + 1) * 512).bitcast(dt.int16),
                                psh[:],
                                float(128.0 * np.log2(np.e)),
                                float(127.0 * 128.0 - 0.043677448 * 128.0),
                                OP.mult,
                                OP.add,
                            )
                        emit_el(s, c)
                    else:
                        ps = p_s.tile([128, w], f32, tag="s", name=f"s{s}_{c}")
                        for n in range(2):
                            nc.tensor.matmul(
                                ps[:, n * 512 : (n + 1) * 512],
                                kt[:, c * 128 : (c + 1) * 128],
                                qt[:, qlo + n * 512 : qlo + (n + 1) * 512],
                                start=True,
                                stop=True,
                            )
                        nc.scalar.activation(
                            ecs(c, qlo, qlo + w), ps[:], AF.Exp
                        )
                        emit_el(s, c)

                # qt seg0 + kt first so scores/exp start early; everything
                # else hides under the Act-bound exp stream
                proj_group(qt, 0, 0, 0, 0, "q0")
                # mini kt group for chunk 0 only: unblocks the very first
                # scores matmul ~0.5us before the full 512-wide group lands
                psm = p_scr.tile([DH, 256], f32, tag="scr", name="ktmini")
                for kc in range(4):
                    nc.tensor.matmul(
                        psm[:],
                        wchunk(kc, 64, 128),
                        xchunk(kc, 0, 256),
                        start=(kc == 0),
                        stop=(kc == 3),
                    )
                nc.scalar.copy(kt[:, 0:256], psm[:])
                scores_exp(0, 0)
                proj_group(kt, 64, 1, 0, 0, "k0", lo0=256)
                for c in range(1, 4):
                    scores_exp(0, c)
                # mini kt group for chunk 4 (panel-1 gated): unblocks
                # scores(0,4) before the full k0b group completes
                psm2 = p_scr.tile([DH, 256], f32, tag="scr", name="ktmini2")
                for kc in range(4):
                    nc.tensor.matmul(
                        psm2[:],
                        wchunk(kc, 64, 128),
                        xchunk(kc, 512, 768),
                        start=(kc == 0),
                        stop=(kc == 3),
                    )
                nc.vector.tensor_copy(kt[:, 512:768], psm2[:])
                scores_exp(0, 4)
                proj_group(kt, 64, 1, 0, 1, "k0b", lo0=256)
                for c in range(5, 8):
                    scores_exp(0, c)
                proj_group(kt, 64, 1, 1, 0, "k1a")
                for c in range(8, 10):
                    scores_exp(0, c)
                # q1a/q1b pulled INSIDE seg0 so the seg0->seg1 boundary has
                # no projection stall on the Act exp stream
                proj_group(qt, 0, 0, 0, 1, "q1a")
                for c in range(10, 12):
                    scores_exp(0, c)
                proj_group(kt, 64, 1, 1, 1, "k1b")
                for c in range(12, 14):
                    scores_exp(0, c)
                    for cc in range(4 * (c - 12), 4 * (c - 12) + 4):
                        v_chunk(cc)
                proj_group(qt, 0, 0, 1, 0, "q1b")
                for c in range(14, NCHUNK):
                    scores_exp(0, c)
                    for cc in range(4 * (c - 12), 4 * (c - 12) + 4):
                        v_chunk(cc)

                def pipe(j, pool):
                    av_front(j, pool)
                    if j >= 1:
                        av_tr(j - 1, pool)
                    if j >= 2:
                        av_out(j - 2, pool)

                # seg1 scores/exp with V and B(seg0) (4 q-chunks) interleaved.
                # fast1 (DVE fast-exp) chunks alternate with Act chunks so the
                # Act exp stream never goes dry while DVE works
                seg1_act = [c for c in range(NCHUNK) if c not in fast1]
                seg1_fast = [c for c in range(NCHUNK) if c in fast1]
                seg1_order = []
                for i, c in enumerate(seg1_act):
                    if seg1_fast:
                        seg1_order.append(seg1_fast.pop(0))
                    seg1_order.append(c)
                for i, c in enumerate(seg1_order):
                    scores_exp(1, c)
                    if i == 10:
                        # q2 inside seg1: with the Act-only p_s ring, the
                        # seg1->seg2 boundary otherwise exposes q2's whole
                        # proj chain as a 4.3us Act stall
                        proj_group(qt, 0, 0, 1, 1, "q2")
                    if i % 4 == 3:
                        pipe(i // 4, p_scr)
                # seg2 scores/exp with B(seg1) (8 q-chunks) interleaved
                for c in range(NCHUNK):
                    scores_exp(2, c)
                    if c % 2 == 1:
                        pipe(4 + c // 2, p_scr)

            # ------------- section 2: B(seg2) with dedicated pools ---------
            with (
                tc.tile_pool(name="av2", bufs=4, space="PSUM") as p_av2,
                tc.tile_pool(name="tr2", bufs=2, space="PSUM") as p_tr2,
                tc.tile_pool(name="op2", bufs=2, space="PSUM") as p_op2,
            ):
                # deep stage-skew: all fronts ASAP so PE never waits on the
                # DVE/Pool chain; backs interleaved to satisfy pool rotation
                av_front(12, p_av2)
                av_front(13, p_av2)
                av_tr(11, p_tr2)
                av_front(14, p_av2)
                av_out(10, p_op2)
                av_tr(12, p_tr2)
                av_front(15, p_av2)
                av_out(11, p_op2)
                av_tr(13, p_tr2)
                av_out(12, p_op2)
                av_tr(14, p_tr2)
                av_out(13, p_op2)
                av_tr(15, p_tr2)
                av_out(14, p_op2)
                av_out(15, p_op2)

    _legalize_waits(nc, mybir)
    return nc


_CACHE = {}


def _get_built(bar_key, bars):
    if bar_key not in _CACHE:
        _CACHE[bar_key] = _build(bars)
    return _CACHE[bar_key]


def _np_reference(hidden_states, bar_positions, attention_mask, Wq, bq, Wk, bk,
                  Wv, bv, Wo, bo, bar_emb, gate):
    """Plain numpy fallback (only used if inputs violate baked assumptions)."""
    B, S_, _ = hidden_states.shape
    x = hidden_states.astype(np.float64)
    q = (x @ Wq.T + bq).reshape(B, S_, H, DH).transpose(0, 2, 1, 3)
    k = (x @ Wk.T + bk).reshape(B, S_, H, DH).transpose(0, 2, 1, 3)
    v = (x @ Wv.T + bv).reshape(B, S_, H, DH).transpose(0, 2, 1, 3)
    scores = np.einsum("bhqd,bhkd->bhqk", q, k) * SCALE
    pad = attention_mask[:, None, None, :]
    bar_mask = (bar_positions[:, :, None] == bar_positions[:, None, :])[:, None]
    NEG = -np.inf

    def softmax(s):
        s = s - s.max(-1, keepdims=True)
        e = np.exp(s)
        return e / e.sum(-1, keepdims=True)

    local = softmax(np.where(bar_mask & pad, scores, NEG))
    emb = bar_emb[np.asarray(bar_positions) % bar_emb.shape[0]]
    bias = np.sum(emb * emb, axis=-1)
    glob = softmax(np.where(pad, scores + bias[:, None, :, None], NEG))
    la = np.einsum("bhqk,bhkd->bhqd", local, v)
    ga = np.einsum("bhqk,bhkd->bhqd", glob, v)
    g = 1.0 / (1.0 + np.exp(-gate))[None, :, None, None]
    comb = g * la + (1.0 - g) * ga
    out = comb.transpose(0, 2, 1, 3).reshape(B, S_, H * DH)
    return (out @ Wo.T + bo).astype(np.float32)


def kernel(**inputs):
    import ml_dtypes

    bf = ml_dtypes.bfloat16
    hidden_states = np.asarray(inputs["hidden_states"], dtype=np.float32)
    bar_positions = np.asarray(inputs["bar_positions"])
    attention_mask = np.asarray(inputs["attention_mask"])
    Wq = np.asarray(inputs["Wq"], dtype=np.float32)
    bq = np.asarray(inputs["bq"], dtype=np.float32)
    Wk = np.asarray(inputs["Wk"], dtype=np.float32)
    bk = np.asarray(inputs["bk"], dtype=np.float32)
    Wv = np.asarray(inputs["Wv"], dtype=np.float32)
    bv = np.asarray(inputs["bv"], dtype=np.float32)
    Wo = np.asarray(inputs["Wo"], dtype=np.float32)
    bo = np.asarray(inputs["bo"], dtype=np.float32)
    gate = np.asarray(inputs["gate"], dtype=np.float32)

    bp = bar_positions[0].astype(np.int64)
    usable = (
        hidden_states.shape == (1, S, D)
        and bool(attention_mask.all())
        and bool((np.diff(bp) >= 0).all())
        and bool((bp >= 0).all())
        and bool((bp < 256).all())
        and not bool(bv.any())  # bias folds not implemented on-device
        and not bool(bq.any())
        and not bool(bk.any())
    )
    if usable:
        bars = _bar_bounds(bp)
        _, albo, albi = _bands(bars)
        usable = all(albi[c] - albo[c] <= 1024 for c in range(NCHUNK))
    if not usable:
        return _np_reference(
            hidden_states, bar_positions, attention_mask, Wq, bq, Wk, bk,
            Wv, bv, Wo, bo, np.asarray(inputs["bar_emb"], dtype=np.float32), gate,
        )

    nc = _get_built(bp.tobytes(), bars)

    # shared inputs
    xt = hidden_states[0].T  # [512, 2048] f32
    xt_pack = np.ascontiguousarray(
        xt.reshape(4, 128, S).transpose(1, 0, 2).reshape(128, 4 * S)
    ).astype(bf)
    ident = np.eye(128, dtype=np.float32).astype(bf)
    widths = [albi[c] - albo[c] for c in range(NCHUNK)]
    maskband = np.zeros((128, sum(widths)), dtype=bf)
    off = 0
    for c in range(NCHUNK):
        eq = bp[c * 128 : (c + 1) * 128, None] == bp[None, albo[c] : albi[c]]
        maskband[:, off : off + widths[c]] = eq.astype(bf)
        off += widths[c]

    g = 1.0 / (1.0 + np.exp(-gate.astype(np.float64)))  # sigmoid, [H]
    in_maps = []
    for h in range(H):
        sl = slice(h * DH, (h + 1) * DH)
        wpack = np.empty((D, 192), dtype=np.float32)
        wpack[:, 0:64] = Wq[sl, :].T * np.float32(SCALE)
        wpack[:, 64:128] = Wk[sl, :].T
        wpack[:, 128:192] = Wv[sl, :].T
        wpack = np.ascontiguousarray(
            wpack.reshape(4, 128, 192).transpose(1, 0, 2).reshape(128, 4 * 192)
        ).astype(bf)
        wot = np.ascontiguousarray(Wo[:, sl].T).astype(bf)  # [64, 512]
        consts = np.zeros((128, 4), dtype=np.float32)
        consts[0:DH, 0] = bq[sl] * np.float32(SCALE)
        consts[0:DH, 1] = bk[sl]
        consts[:, 2] = np.float32(1.0 / g[h])
        consts[:, 3] = np.float32(1.0 / (1.0 - g[h]))
        in_maps.append(
            {"xt": xt_pack, "wpack": wpack, "wot": wot,
             "maskband": maskband, "ident": ident, "consts": consts}
        )

    res = _run_spmd(nc, in_maps)
    out = np.zeros((S, D), dtype=np.float32)
    for h in range(H):
        out += np.asarray(res.results[h]["out_partial"]).astype(np.float32)
    out += bo
    return out.reshape(1, S, D)


def _run_spmd(nc, in_maps, **kw):
    from concourse.bass_utils import run_bass_kernel_spmd

    return run_bass_kernel_spmd(nc, in_maps, list(range(H)), **kw)

